# revision 25
# baseline (speedup 1.0000x reference)
"""Trainium2 Bass kernel for Exphormer-style sparse graph attention.

Math (per reference):
  Q = x @ Wq ; K = x @ Wk ; V = x @ Wv          (biases are zero; [N, H, D])
  dot[e]   = sum_d K[src[e]] * Q[dst[e]] / sqrt(D)
  score[e] = exp(clip(dot, -5, 5))
  out[n]   = (sum_{e:dst=n} V[src[e]]*score[e]) / (sum_{e:dst=n} score[e] + 1e-6)

Distribution: destination-sharded across 8 cores, no collectives.
Core c owns dst nodes [c*N/8, (c+1)*N/8).

v2 design (vs v1): bf16 tables/gather/matmuls (validated ~4e-3 fro err on
host), dst bands of B=128 nodes (pages), pages processed in groups of G.
Edges per group are laid out in 4 chunk segments (kv_table split in 4 so
row indices fit dma_gather's int16), with *static* per-(page,chunk) offsets
(max edge count over all 8 cores) so the one SPMD program works for every
core.  K|V rows are fetched with 4 bulk dma_gather calls per group and Q
rows (local dst ids < 32k, no chunking) with one more; the dst-scatter is
one-hot matmuls into per-band PSUM accumulators.  A tile whose 128 edge
slots straddle a page boundary gets one scatter matmul per touched band
(host emits the static schedule; pad slots carry slot=-1 and vanish from
every one-hot).
"""

import os
import sys
from dataclasses import dataclass, field

import numpy as np

for _p in ("/opt/trn_rl_repo", os.path.expanduser("~/trn_rl_repo")):
    if os.path.isdir(_p) and _p not in sys.path:
        sys.path.insert(0, _p)

os.environ.setdefault("MYCRO_LOCAL_CACHE", "1")

import ml_dtypes  # noqa: E402

import concourse.bass as bass  # noqa: E402
import concourse.tile as tile  # noqa: E402
from concourse import bacc, mybir  # noqa: E402
from concourse.bass_utils import run_bass_kernel_spmd  # noqa: E402

F32 = mybir.dt.float32
BF16 = mybir.dt.bfloat16
I16 = mybir.dt.int16
AF = mybir.ActivationFunctionType
OP = mybir.AluOpType

P = 128
CLIP = 5.0
BF = ml_dtypes.bfloat16


@dataclass(frozen=True)
class Params:
    n_nodes: int = 100000
    in_dim: int = 128
    heads: int = 8
    head_dim: int = 16
    n_cores: int = 8
    group: int = 5  # pages (dst bands of 128) per processing group
    n_chunks: int = 4  # kv_table row-index chunks (dma_gather int16 limit)

    @property
    def npc(self):
        return self.n_nodes // self.n_cores  # 12500

    @property
    def n_pages(self):
        return (self.npc + P - 1) // P  # 98

    @property
    def out_rows(self):
        return self.n_pages * P  # 12544

    @property
    def kv_rows(self):
        return ((self.n_nodes + P - 1) // P) * P  # 100096

    @property
    def chunk_rows(self):
        # tile-aligned so each chunk is its own DRAM tensor (per-chunk
        # dependency tracking lets gathers start before the whole kv
        # projection finishes); last chunk is shorter.
        return 196 * P  # 25088

    def chunk_size(self, c):
        return min(self.chunk_rows, self.kv_rows - c * self.chunk_rows)

    @property
    def fdim(self):
        return self.heads * self.head_dim  # 128


PARAMS = Params()


@dataclass
class Layout:
    """Static (core-independent) edge layout + scatter schedule."""

    # per group: list of per-chunk tile counts [NG][CH]
    seg_tiles: list = field(default_factory=list)
    # per group: page list (global page ids)
    group_pages: list = field(default_factory=list)
    # per group: scatter entries (chunk, local_tile, band_local, use_hi,
    #            start, stop) in emission order
    schedule: list = field(default_factory=list)
    # static per (page, chunk) run offset within its group segment
    run_off: np.ndarray | None = None  # [98, CH]
    run_len: np.ndarray | None = None  # [98, CH] (max count over cores)
    nt_total: int = 0  # total tiles per core

    @property
    def n_groups(self):
        return len(self.seg_tiles)


def build_layout(cnt_max: np.ndarray, prm: Params) -> Layout:
    """cnt_max: [n_pages, n_chunks] max-over-cores edge counts."""
    lay = Layout()
    NPg, CH, G = prm.n_pages, prm.n_chunks, prm.group
    lay.run_off = np.zeros((NPg, CH), np.int64)
    lay.run_len = cnt_max.astype(np.int64)
    for g0 in range(0, NPg, G):
        pages = list(range(g0, min(g0 + G, NPg)))
        seg_t = []
        for c in range(CH):
            off = 0
            for p in pages:
                lay.run_off[p, c] = off
                off += int(cnt_max[p, c])
            seg_t.append((off + P - 1) // P)
        lay.group_pages.append(pages)
        lay.seg_tiles.append(seg_t)

        # scatter schedule: which (chunk, tile) touches which band
        entries = []  # (chunk, local_tile, band_local, use_hi)
        for c in range(CH):
            # band of every slot position in this segment
            span = seg_t[c] * P
            pos_band = np.full(span, -1, np.int64)
            for bi, p in enumerate(pages):
                o, l = int(lay.run_off[p, c]), int(cnt_max[p, c])
                pos_band[o : o + l] = bi
            for t in range(seg_t[c]):
                bands = sorted(
                    {int(b) for b in pos_band[t * P : (t + 1) * P] if b >= 0}
                )
                assert len(bands) <= 2, f"3-band tile: {bands}"
                for k, b in enumerate(bands):
                    entries.append((c, t, b, k == 1))
        # start/stop flags per band
        first_of = {}
        last_of = {}
        for i, (c, t, b, hi) in enumerate(entries):
            first_of.setdefault(b, i)
            last_of[b] = i
        sched = [
            (c, t, b, hi, i == first_of[b], i == last_of[b])
            for i, (c, t, b, hi) in enumerate(entries)
        ]
        lay.schedule.append(sched)
    lay.nt_total = sum(sum(s) for s in lay.seg_tiles)
    return lay


def wrap16(idx: np.ndarray) -> np.ndarray:
    """[n] -> [128, n//16] int16: idx i at [i%16, i//16], replicated x8."""
    n = len(idx)
    a = np.zeros((16, n // 16), np.int16)
    a[np.arange(n) % 16, np.arange(n) // 16] = idx.astype(np.int16)
    return np.tile(a, (8, 1))


def preprocess(x, edge_index, wq, wk, wv, prm: Params):
    src_a = np.asarray(edge_index[0], np.int64)
    dst_a = np.asarray(edge_index[1], np.int64)
    order = np.argsort(dst_a, kind="stable")
    s_src = src_a[order]
    s_dst = dst_a[order]
    core_bounds = np.searchsorted(
        s_dst, np.arange(0, prm.n_nodes + 1, prm.npc, dtype=np.int64)
    )

    NPg, CH = prm.n_pages, prm.n_chunks
    per_core = []
    cnt_max = np.zeros((NPg, CH), np.int64)
    for c in range(prm.n_cores):
        cs, ce = core_bounds[c], core_bounds[c + 1]
        d = s_dst[cs:ce] - c * prm.npc
        s = s_src[cs:ce]
        pg = d // P
        ck = s // prm.chunk_rows
        cnt = np.zeros((NPg, CH), np.int64)
        np.add.at(cnt, (pg, ck), 1)
        cnt_max = np.maximum(cnt_max, cnt)
        per_core.append((d, s, pg, ck, cnt))

    lay = build_layout(cnt_max, prm)
    NT = lay.nt_total

    # projection inputs (shared across cores except xTl)
    xT = np.zeros((prm.in_dim, prm.kv_rows), BF)
    xT[:, : prm.n_nodes] = np.asarray(x, np.float32).T.astype(BF)
    wkv = np.concatenate(
        [np.asarray(wk, np.float32), np.asarray(wv, np.float32)], axis=1
    ).astype(BF)
    wqb = np.asarray(wq, np.float32).astype(BF)
    iota_row = np.broadcast_to(
        np.arange(P, dtype=np.float32), (P, P)
    ).astype(BF).copy()

    in_maps = []
    for c in range(prm.n_cores):
        d, s, pg, ck, cnt = per_core[c]
        # position of each edge inside its (page, chunk) run
        run_base = np.zeros((NPg, CH), np.int64)
        # order edges by (page, chunk) to compute within-run ranks
        key = pg * CH + ck
        eorder = np.argsort(key, kind="stable")
        ks = key[eorder]
        # rank within equal keys
        first_idx = np.r_[0, np.flatnonzero(np.diff(ks)) + 1]
        starts = np.zeros(len(ks), np.int64)
        starts[first_idx] = 1
        grp_id = np.cumsum(starts) - 1
        rank = np.arange(len(ks)) - first_idx[grp_id]
        pos_in_run = np.empty(len(ks), np.int64)
        pos_in_run[eorder] = rank

        # global slot of each edge
        group_starts = np.array(
            [pp[0] for pp in lay.group_pages] + [NPg], np.int64
        )
        gi = np.searchsorted(group_starts, pg, "right") - 1
        # tile base of (group, chunk) in global tile index
        gb = np.zeros(lay.n_groups + 1, np.int64)
        for g in range(lay.n_groups):
            gb[g + 1] = gb[g] + sum(lay.seg_tiles[g])
        seg_base = np.zeros((lay.n_groups, CH), np.int64)
        for g in range(lay.n_groups):
            o = 0
            for cc in range(CH):
                seg_base[g, cc] = gb[g] + o
                o += lay.seg_tiles[g][cc]
        slot_global = (
            seg_base[gi, ck] * P + lay.run_off[pg, ck] + pos_in_run
        )

        kvidx = np.zeros(NT * P, np.int64)
        qidx = np.zeros(NT * P, np.int64)
        slot_lo = np.full(NT * P, -1.0, np.float32)
        slot_hi = np.full(NT * P, -1.0, np.float32)
        kvidx[slot_global] = s - ck * prm.chunk_rows
        qidx[slot_global] = d

        # lo/hi band assignment is static per tile; edge band = its page
        tile_of = slot_global // P
        band_local = pg - group_starts[gi]
        # per tile: low band = min band among schedule entries
        tile_lo = np.full(NT, -2, np.int64)
        for g in range(lay.n_groups):
            for cc in range(CH):
                span = lay.seg_tiles[g][cc]
                base = seg_base[g, cc]
                ents = [
                    e for e in lay.schedule[g] if e[0] == cc
                ]
                for (ecc, t, b, hi, st, sp) in ents:
                    if not hi:
                        tile_lo[base + t] = b
        is_lo = band_local == tile_lo[tile_of]
        sv = (d - pg * P).astype(np.float32)
        slot_lo[slot_global[is_lo]] = sv[is_lo]
        slot_hi[slot_global[~is_lo]] = sv[~is_lo]

        # SBUF layouts
        def seg_slices(arr):
            """concat per (g, ch) wrap16 of the segment's idx list."""
            out = []
            for g in range(lay.n_groups):
                for cc in range(CH):
                    b = seg_base[g, cc] * P
                    e = b + lay.seg_tiles[g][cc] * P
                    out.append(wrap16(arr[b:e]))
            return np.concatenate(out, axis=1)

        kvidx16 = seg_slices(kvidx)
        # q idx wraps per GROUP (one gather per group over all 4 segments)
        qout = []
        for g in range(lay.n_groups):
            b = gb[g] * P
            e = gb[g + 1] * P
            qout.append(wrap16(qidx[b:e]))
        qidx16 = np.concatenate(qout, axis=1)

        slot_lo_sb = np.ascontiguousarray(
            slot_lo.reshape(NT, P).T
        ).astype(BF)
        slot_hi_sb = np.ascontiguousarray(
            slot_hi.reshape(NT, P).T
        ).astype(BF)

        xTl = np.zeros((prm.in_dim, prm.out_rows), BF)
        xTl[:, : prm.npc] = (
            np.asarray(x[c * prm.npc : (c + 1) * prm.npc], np.float32)
            .T.astype(BF)
        )

        in_maps.append(
            {
                "xT": xT,
                "xTl": xTl,
                "wkv": wkv,
                "wq": wqb,
                "iota_row": iota_row,
                "kvidx": kvidx16,
                "qidx": qidx16,
                "slot_lo": slot_lo_sb,
                "slot_hi": slot_hi_sb,
            }
        )
    return in_maps, lay


def build_program(prm: Params, lay: Layout):
    nc = bacc.Bacc(
        "TRN2", target_bir_lowering=False, debug=False, num_swdge_queues=4
    )
    C = prm.in_dim
    F = prm.fdim
    F2 = 2 * F
    H, D = prm.heads, prm.head_dim
    CH = prm.n_chunks
    NT = lay.nt_total
    PAYW = F + H  # 136

    xT = nc.declare_dram_parameter("xT", [C, prm.kv_rows], BF16, False)
    xTl = nc.declare_dram_parameter("xTl", [C, prm.out_rows], BF16, False)
    wkv = nc.declare_dram_parameter("wkv", [C, F2], BF16, False)
    wq = nc.declare_dram_parameter("wq", [C, F], BF16, False)
    iota_row = nc.declare_dram_parameter("iota_row", [P, P], BF16, False)
    kvidx = nc.declare_dram_parameter("kvidx", [P, NT * 8], I16, False)
    qidx = nc.declare_dram_parameter("qidx", [P, NT * 8], I16, False)
    slot_lo = nc.declare_dram_parameter("slot_lo", [P, NT], BF16, False)
    slot_hi = nc.declare_dram_parameter("slot_hi", [P, NT], BF16, False)
    out = nc.declare_dram_parameter("out", [prm.out_rows, F], F32, True)

    kv_chunks = [
        nc.dram_tensor(f"kv_chunk{c}", [prm.chunk_size(c), F2], BF16)
        for c in range(CH)
    ]
    q_table = nc.dram_tensor("q_table", [prm.out_rows, F], BF16)

    n_q_tiles = prm.out_rows // P  # 98
    GL = 8

    tgc_max = max(max(s) for s in lay.seg_tiles)
    tg_max = max(sum(s) for s in lay.seg_tiles)

    with tile.TileContext(nc) as tc:
        with (
            tc.tile_pool(name="const", bufs=1) as cpool,
            tc.tile_pool(name="proj", bufs=3) as ppool,
            tc.tile_pool(name="gath", bufs=2) as gpool,
            tc.tile_pool(name="mid", bufs=2) as mpool,
            tc.tile_pool(name="uni", bufs=1) as upool,
            tc.tile_pool(name="small", bufs=2) as spool,
            tc.tile_pool(name="oh", bufs=6) as ohpool,
            tc.tile_pool(name="psum_p", bufs=2, space="PSUM") as psp,
            tc.tile_pool(name="psum_a", bufs=1, space="PSUM") as psa,
        ):
            wkv_sb = cpool.tile([C, F2], BF16)
            nc.sync.dma_start(out=wkv_sb[:], in_=wkv[:])
            wq_sb = cpool.tile([C, F], BF16)
            nc.sync.dma_start(out=wq_sb[:], in_=wq[:])
            ir_sb = cpool.tile([P, P], BF16)
            nc.sync.dma_start(out=ir_sb[:], in_=iota_row[:])
            kvidx_sb = cpool.tile([P, NT * 8], I16)
            nc.sync.dma_start(out=kvidx_sb[:], in_=kvidx[:])
            qidx_sb = cpool.tile([P, NT * 8], I16)
            nc.sync.dma_start(out=qidx_sb[:], in_=qidx[:])
            slo_sb = cpool.tile([P, NT], BF16)
            nc.sync.dma_start(out=slo_sb[:], in_=slot_lo[:])
            shi_sb = cpool.tile([P, NT], BF16)
            nc.sync.dma_start(out=shi_sb[:], in_=slot_hi[:])

            copy_tick = [0]

            def stage_copy(out_ap, in_ap):
                # PSUM-drain copies: 1/4 DVE, 3/4 ACT (DVE is the busier
                # engine during the edge phase)
                if copy_tick[0] % 4 == 0:
                    nc.vector.tensor_copy(out=out_ap, in_=in_ap)
                else:
                    nc.scalar.copy(out=out_ap, in_=in_ap)
                copy_tick[0] += 1

            def project(src_ap, w_ap, table, n_tiles, fw):
                n_groups = (n_tiles + GL - 1) // GL
                tbl = table[:].rearrange("(t p) f -> t p f", p=P)
                for g in range(n_groups):
                    k_here = min(GL, n_tiles - g * GL)
                    cols = k_here * P
                    xt_g = ppool.tile([C, GL * P], BF16, tag="xt_g")
                    nc.sync.dma_start(
                        out=xt_g[:, :cols],
                        in_=src_ap[:, g * GL * P : g * GL * P + cols],
                    )
                    stage = ppool.tile([P, GL, fw], BF16, tag="stage")
                    for k in range(k_here):
                        ps = psp.tile([P, fw], F32, tag="ps")
                        nc.tensor.matmul(
                            out=ps[:],
                            lhsT=xt_g[:, k * P : (k + 1) * P],
                            rhs=w_ap,
                            start=True,
                            stop=True,
                        )
                        stage_copy(stage[:, k, :], ps[:])
                    if k_here == GL:
                        view = table[
                            g * GL * P : (g + 1) * GL * P, :
                        ].rearrange("(k p) f -> p k f", p=P)
                        nc.sync.dma_start(out=view, in_=stage[:])
                    else:
                        for k in range(k_here):
                            nc.sync.dma_start(
                                out=tbl[g * GL + k], in_=stage[:, k, :]
                            )

            # q first so q gathers can start early; kv per chunk so each
            # chunk's gathers only wait on that chunk's projection.
            project(xTl[:], wq_sb[:], q_table, n_q_tiles, F)
            for c in range(CH):
                project(
                    xT[:, c * prm.chunk_rows : c * prm.chunk_rows
                       + prm.chunk_size(c)],
                    wkv_sb[:],
                    kv_chunks[c],
                    prm.chunk_size(c) // P,
                    F2,
                )

            # tile bases
            gb = [0]
            for g in range(lay.n_groups):
                gb.append(gb[-1] + sum(lay.seg_tiles[g]))
            qrot = [0]

            for g in range(lay.n_groups):
                segs = lay.seg_tiles[g]
                tg = sum(segs)
                pages = lay.group_pages[g]
                nb = len(pages)
                base = gb[g]

                # HW ucode caps one dma_gather at ~1024 indices; split into
                # windows of <= 8 tiles, round-robined over 4 SWDGE queues
                # (parallel descriptor generation, ~3x).
                GW = 8
                kv_g = gpool.tile([P, tg_max, F2], BF16, tag="kv_g")
                off = 0
                for c in range(CH):
                    tcnt = segs[c]
                    for w0 in range(0, tcnt, GW):
                        w = min(GW, tcnt - w0)
                        ib = (base + off + w0) * 8
                        nc.gpsimd.dma_gather(
                            kv_g[:, off + w0 : off + w0 + w, :],
                            kv_chunks[c][:],
                            kvidx_sb[:, ib : ib + w * 8],
                            w * P,
                            w * P,
                            F2,
                            queue_num=qrot[0] % 4,
                        )
                        qrot[0] += 1
                    off += tcnt
                qe_g = gpool.tile([P, tg_max, F], BF16, tag="qe_g")
                for w0 in range(0, tg, GW):
                    w = min(GW, tg - w0)
                    ib = (base + w0) * 8
                    nc.gpsimd.dma_gather(
                        qe_g[:, w0 : w0 + w, :],
                        q_table[:],
                        qidx_sb[:, ib : ib + w * 8],
                        w * P,
                        w * P,
                        F,
                        queue_num=qrot[0] % 4,
                    )
                    qrot[0] += 1

                prod = upool.tile([P, tg_max, F], BF16, tag="prod")
                nc.vector.tensor_tensor(
                    out=prod[:, :tg, :],
                    in0=kv_g[:, :tg, 0:F],
                    in1=qe_g[:, :tg, :],
                    op=OP.mult,
                )
                dot = spool.tile([P, tg_max, H], F32, tag="dot")
                nc.vector.tensor_reduce(
                    out=dot[:, :tg, :],
                    in_=prod[:, :tg, :].rearrange(
                        "p k (h d) -> p k h d", d=D
                    ),
                    axis=mybir.AxisListType.X,
                    op=OP.add,
                )
                # exp(clip(x,-20,20)/4) == clamp(exp(x/4), e^-5, e^5)
                score_r = spool.tile([P, tg_max, H], BF16, tag="score_r")
                nc.scalar.activation(
                    out=score_r[:, :tg, :],
                    in_=dot[:, :tg, :],
                    func=AF.Exp,
                    scale=0.25,
                )
                score = spool.tile([P, tg_max, H], BF16, tag="score")
                nc.vector.tensor_scalar(
                    out=score[:, :tg, :],
                    in0=score_r[:, :tg, :],
                    scalar1=float(np.exp(CLIP)),
                    scalar2=float(np.exp(-CLIP)),
                    op0=OP.min,
                    op1=OP.max,
                )
                payload = mpool.tile([P, tg_max, PAYW], BF16, tag="payload")
                nc.vector.tensor_tensor(
                    out=payload[:, :tg, 0:F].rearrange(
                        "p k (h d) -> p k h d", d=D
                    ),
                    in0=kv_g[:, :tg, F:F2].rearrange(
                        "p k (h d) -> p k h d", d=D
                    ),
                    in1=score[:, :tg, :]
                    .unsqueeze(3)
                    .to_broadcast([P, tg, H, D]),
                    op=OP.mult,
                )
                nc.scalar.copy(
                    out=payload[:, :tg, F:PAYW], in_=score[:, :tg, :]
                )

                # one-hot scatter masks: whole-group upfront builds (one
                # DVE op each for the lo and hi slot arrays) so the PE
                # scatter loop never waits on DVE mid-stream.
                oh_lo = ohpool.tile(
                    [P, tg_max, P], BF16, tag="oh_lo", bufs=1
                )
                nc.vector.tensor_tensor(
                    out=oh_lo[:, :tg, :],
                    in0=ir_sb[:].unsqueeze(1).to_broadcast([P, tg, P]),
                    in1=slo_sb[:, base : base + tg]
                    .unsqueeze(2)
                    .to_broadcast([P, tg, P]),
                    op=OP.is_equal,
                )
                seg_off = np.cumsum([0] + list(segs))
                hi_oh = {}
                for (c, t, b, hi, st, sp) in lay.schedule[g]:
                    if not hi:
                        continue
                    gt = base + seg_off[c] + t
                    oh = ohpool.tile(
                        [P, P], BF16, tag=f"oh{len(hi_oh) % 24}",
                        name="oh", bufs=1,
                    )
                    nc.vector.tensor_tensor(
                        out=oh[:],
                        in0=ir_sb[:],
                        in1=shi_sb[:, gt : gt + 1].to_broadcast([P, P]),
                        op=OP.is_equal,
                    )
                    hi_oh[(c, t)] = oh

                acc_ps = {
                    b: psa.tile(
                        [P, PAYW], F32, tag=f"acc{b}", name=f"acc{b}"
                    )
                    for b in range(nb)
                }
                for (c, t, b, hi, st, sp) in lay.schedule[g]:
                    lhs = (
                        hi_oh[(c, t)][:]
                        if hi
                        else oh_lo[:, seg_off[c] + t, :]
                    )
                    nc.tensor.matmul(
                        out=acc_ps[b][:],
                        lhsT=lhs,
                        rhs=payload[:, seg_off[c] + t, :],
                        start=st,
                        stop=sp,
                    )

                outstage = spool.tile(
                    [P, prm.group, F], F32, tag="outstage"
                )
                for b in range(nb):
                    zr = spool.tile([P, H], F32, tag="zr")
                    nc.vector.tensor_scalar_add(
                        out=zr[:], in0=acc_ps[b][:, F:PAYW], scalar1=1e-6
                    )
                    zri = spool.tile([P, H], F32, tag="zri")
                    nc.vector.reciprocal(out=zri[:], in_=zr[:])
                    nc.vector.tensor_tensor(
                        out=outstage[:, b, :].rearrange(
                            "p (h d) -> p h d", d=D
                        ),
                        in0=acc_ps[b][:, 0:F].rearrange(
                            "p (h d) -> p h d", d=D
                        ),
                        in1=zri[:].unsqueeze(2).to_broadcast([P, H, D]),
                        op=OP.mult,
                    )
                r0 = pages[0] * P
                nc.sync.dma_start(
                    out=out[r0 : r0 + nb * P, :].rearrange(
                        "(b p) f -> p b f", p=P
                    ),
                    in_=outstage[:, :nb, :],
                )
    nc.compile()
    return nc


def run(inputs: dict, prm: Params = PARAMS, **run_kwargs):
    bq = np.asarray(inputs["bq"])
    bk = np.asarray(inputs["bk"])
    bv = np.asarray(inputs["bv"])
    assert not (np.any(bq) or np.any(bk) or np.any(bv)), (
        "nonzero projection biases not supported by this kernel build"
    )
    in_maps, lay = preprocess(
        inputs["x"], inputs["edge_index"], inputs["Wq"], inputs["Wk"],
        inputs["Wv"], prm,
    )
    nc = build_program(prm, lay)
    res = run_bass_kernel_spmd(
        nc, in_maps, core_ids=list(range(prm.n_cores)), **run_kwargs
    )
    return res, in_maps


def kernel(**inputs) -> np.ndarray:
    prm = PARAMS
    res, _ = run(inputs, prm)
    shards = [res.results[c]["out"][: prm.npc] for c in range(prm.n_cores)]
    return np.concatenate(shards, axis=0).astype(np.float32)
